# revision 2
# baseline (speedup 1.0000x reference)
"""BiLSTM classifier Trainium2 kernel (8 NeuronCores, SPMD).

Model (reference): emb = table[x]; c_f = LSTM_final_cell(emb, fwd);
c_b = LSTM_final_cell(flip(emb), bwd); out = [c_f, c_b] @ Wd + bd.

Sharding: 8 cores = 2 directions x 4 batch-shards of 64 rows; each core runs
2 interleaved independent LSTM "chains" of batch 32 (fills engine idle time of
the serial recurrence). All state is kept TRANSPOSED on-chip:
hidden/gates on partitions, batch along the free dim, so the per-step
recurrent matmul streams only N=32 columns and the elementwise/activation ops
use all 128 lanes. Per step (per chain):
  z^T[128, 8*32] (PSUM) = I.T @ xW_t^T (start=True inject)  +
                          sum_k Wh[k,m]^T @ h^T[k]  (16 accumulating matmuls)
  sg = sigmoid(z)  (one ACT op; tanh folded to sigmoid via host 2x weight scales)
  t1 = f*c ; t2 = (sg_g - 0.5)*i ; c = 2*t2 + t1      (DVE, fused stt ops)
  sc = sigmoid(2c) ; h' = (sc - 0.5)*o   (h' = h/2; compensated by 2x on Wh)
xW^T is produced on-core per 16-step block: indirect-DMA gather of embedding
rows -> PE transpose -> xW^T = Wx^T-tiles @ emb^T with bias folded in the
PSUM->SBUF copy. Final: partial logits (4 x 32) = Wd_half^T @ c per chain,
summed across direction pairs on the host.
"""

import sys

for _p in ("/opt/trn_rl_repo", "/root/.axon_site/_ro/trn_rl_repo"):
    if _p not in sys.path:
        sys.path.insert(0, _p)

import numpy as np
import ml_dtypes

# ---- problem constants (hardcoded; kernel.py must be self-contained) ----
VOCAB = 32000
EMBED = 128
HIDDEN = 256
NUM_CLASSES = 4
B_FULL, T_FULL = 256, 512

N_CORES = 8
CHAINS = 2          # independent LSTM chains per core
B = 32              # batch per chain
STEPS = 16          # time steps per iteration block
N_ITERS = T_FULL // STEPS
GB = 8 * B          # gate-row block per step in z^T layout ( = 4H/128 * B )
W_NP = ml_dtypes.bfloat16   # on-chip matmul operand dtype

_CACHE = {}


def _build_program():
    import concourse.bacc as bacc
    import concourse.mybir as mybir
    from concourse import bass
    from concourse.tile import TileContext

    f32 = mybir.dt.float32
    i32 = mybir.dt.int32
    wdt = mybir.dt.bfloat16
    SIG = mybir.ActivationFunctionType.Sigmoid
    MULT = mybir.AluOpType.mult
    ADD = mybir.AluOpType.add
    SUB = mybir.AluOpType.subtract

    nc = bacc.Bacc("TRN2", target_bir_lowering=False, debug=False,
                   num_devices=N_CORES)

    # ---- DRAM I/O ----
    emb_dram = nc.dram_tensor("emb", [VOCAB, EMBED], f32, kind="ExternalInput")
    whT_dram = nc.dram_tensor("whT", [128, 16 * 128], wdt, kind="ExternalInput")
    wxT_dram = nc.dram_tensor("wxT", [128, 8 * 128], wdt, kind="ExternalInput")
    bT_dram = nc.dram_tensor("bT", [128, 8], f32, kind="ExternalInput")
    wdT_dram = nc.dram_tensor("wdT", [128, 8], f32, kind="ExternalInput")
    idf_dram = nc.dram_tensor("identf", [128, 128], f32, kind="ExternalInput")
    idw_dram = nc.dram_tensor("identw", [128, 128], wdt, kind="ExternalInput")
    idx_dram = nc.dram_tensor("idx", [N_ITERS, 128, 2 * 4], i32,
                              kind="ExternalInput")
    out_dram = nc.dram_tensor("out", [CHAINS, NUM_CLASSES, B], f32,
                              kind="ExternalOutput")

    with TileContext(nc) as tc:
        with (
            tc.tile_pool(name="const", bufs=1) as constp,
            tc.tile_pool(name="state", bufs=1) as statep,
            tc.tile_pool(name="idxp", bufs=2) as idxp,
            tc.tile_pool(name="embp", bufs=8) as embp,
            tc.tile_pool(name="embTp", bufs=2) as embTp,
            tc.tile_pool(name="xwp", bufs=2) as xwp,
            tc.tile_pool(name="sgp", bufs=2) as sgp,
            tc.tile_pool(name="tmpp", bufs=2) as tmpp,
            tc.tile_pool(name="outp", bufs=1) as outp,
            tc.tile_pool(name="zps0", bufs=2, space="PSUM") as zps0,
            tc.tile_pool(name="zps1", bufs=2, space="PSUM") as zps1,
            tc.tile_pool(name="xwps", bufs=2, space="PSUM") as xwps,
            tc.tile_pool(name="trps", bufs=1, space="PSUM") as trps,
            tc.tile_pool(name="dps", bufs=1, space="PSUM") as dps,
        ):
            zps = [zps0, zps1]

            # ---- load constants ----
            whT = constp.tile([128, 16 * 128], wdt)
            wxT = constp.tile([128, 8 * 128], wdt)
            bT = constp.tile([128, 8], f32)
            wdT = constp.tile([128, 8], f32)
            idf = constp.tile([128, 128], f32)
            idw = constp.tile([128, 128], wdt)
            for dst, src in ((whT, whT_dram), (wxT, wxT_dram), (bT, bT_dram),
                             (wdT, wdT_dram), (idf, idf_dram), (idw, idw_dram)):
                nc.sync.dma_start(out=dst[:], in_=src[:])

            # ---- per-chain persistent state ----
            hT = [statep.tile([128, 2 * B], wdt, tag=f"hT{c}",
                              name=f"hT{c}") for c in range(CHAINS)]
            cst = [statep.tile([128, 2 * B], f32, tag=f"c{c}",
                               name=f"cst{c}") for c in range(CHAINS)]
            for c in range(CHAINS):
                nc.vector.memset(hT[c][:], 0.0)
                nc.vector.memset(cst[c][:], 0.0)

            for it in range(N_ITERS):
                # ---- gather + transpose emb block; build xW^T for 16 steps --
                idx_sb = idxp.tile([128, 2 * 4], i32)
                nc.sync.dma_start(out=idx_sb[:], in_=idx_dram[it])
                xwT = []
                for c in range(CHAINS):
                    embT = embTp.tile([128, 4 * 128], wdt, tag=f"embT{c}",
                                      name=f"embT{c}")
                    for j in range(4):
                        et = embp.tile([128, 128], f32, tag=f"emb{c}{j}",
                                       name=f"emb{c}{j}")
                        nc.gpsimd.indirect_dma_start(
                            out=et[:], out_offset=None, in_=emb_dram[:],
                            in_offset=bass.IndirectOffsetOnAxis(
                                ap=idx_sb[:, c * 4 + j:c * 4 + j + 1], axis=0),
                        )
                        tp = trps.tile([128, 128], f32)
                        nc.tensor.transpose(out=tp[:], in_=et[:],
                                            identity=idf[:])
                        nc.vector.tensor_copy(
                            out=embT[:, j * 128:(j + 1) * 128], in_=tp[:])
                    xw = xwp.tile([128, STEPS * GB], wdt, tag=f"xw{c}",
                                  name=f"xw{c}")
                    xw4 = xw[:].rearrange("p (s m j) -> p s m j",
                                          s=STEPS, m=8, j=B)
                    for m in range(8):
                        xps = xwps.tile([128, STEPS * B], f32)
                        nc.tensor.matmul(
                            out=xps[:], lhsT=wxT[:, m * 128:(m + 1) * 128],
                            rhs=embT[:], start=True, stop=True)
                        nc.vector.tensor_scalar(
                            out=xw4[:, :, m, :],
                            in0=xps[:].rearrange("p (s j) -> p s j", s=STEPS),
                            scalar1=bT[:, m:m + 1], scalar2=None, op0=ADD)
                    xwT.append(xw)

                # ---- 16 recurrence steps, chains interleaved ----
                for s in range(STEPS):
                    for c in range(CHAINS):
                        z = zps[c].tile([128, GB], f32, tag=f"z{c}",
                                        name=f"z{c}")
                        nc.tensor.matmul(
                            out=z[:], lhsT=idw[:],
                            rhs=xwT[c][:, s * GB:(s + 1) * GB],
                            start=True, stop=False, skip_group_check=True)
                        for m in range(8):
                            for k in range(2):
                                nc.tensor.matmul(
                                    out=z[:, m * B:(m + 1) * B],
                                    lhsT=whT[:, (m * 2 + k) * 128:
                                             (m * 2 + k + 1) * 128],
                                    rhs=hT[c][:, k * B:(k + 1) * B],
                                    start=False, stop=(m == 7 and k == 1),
                                    skip_group_check=True)
                        sg = sgp.tile([128, GB], f32, tag=f"sg{c}",
                                      name=f"sg{c}")
                        nc.scalar.activation(out=sg[:], in_=z[:], func=SIG)
                        t1 = tmpp.tile([128, 2 * B], f32, tag=f"t1{c}",
                                        name=f"t1{c}")
                        t2 = tmpp.tile([128, 2 * B], f32, tag=f"t2{c}",
                                        name=f"t2{c}")
                        sc = tmpp.tile([128, 2 * B], f32, tag=f"sc{c}",
                                        name=f"sc{c}")
                        # t1 = f*c ; t2 = (sig_g-0.5)*i ; c = 2*t2 + t1
                        nc.vector.tensor_mul(
                            out=t1[:], in0=sg[:, 2 * B:4 * B], in1=cst[c][:])
                        nc.vector.scalar_tensor_tensor(
                            out=t2[:], in0=sg[:, 4 * B:6 * B], scalar=0.5,
                            in1=sg[:, 0:2 * B], op0=SUB, op1=MULT)
                        nc.vector.scalar_tensor_tensor(
                            out=cst[c][:], in0=t2[:], scalar=2.0,
                            in1=t1[:], op0=MULT, op1=ADD)
                        # sc = sigmoid(2c); h' = (sc-0.5)*o  (h'=h/2; Wh is 2x)
                        nc.scalar.activation(out=sc[:], in_=cst[c][:],
                                             func=SIG, scale=2.0)
                        nc.vector.scalar_tensor_tensor(
                            out=hT[c][:], in0=sc[:], scalar=0.5,
                            in1=sg[:, 6 * B:8 * B], op0=SUB, op1=MULT)

            # ---- dense epilogue: partial logits = (Wd_half)^T @ c ----
            for c in range(CHAINS):
                dp = dps.tile([NUM_CLASSES, B], f32)
                for k in range(2):
                    nc.tensor.matmul(
                        out=dp[:], lhsT=wdT[:, k * 4:(k + 1) * 4],
                        rhs=cst[c][:, k * B:(k + 1) * B],
                        start=(k == 0), stop=(k == 1))
                ob = outp.tile([NUM_CLASSES, B], f32, tag=f"ob{c}",
                               name=f"ob{c}")
                nc.vector.tensor_copy(out=ob[:], in_=dp[:])
                nc.sync.dma_start(out=out_dram[c], in_=ob[:])

    nc.compile()
    return nc


def _prep_core_inputs(core, x, emb_np, Wx, Wh, b, Wd):
    """Host-side prep: weight layout/scaling + gather index schedule."""
    d, s = core // 4, core % 4
    Wx = Wx.astype(np.float32).copy()
    Wh = Wh.astype(np.float32).copy()
    b = b.astype(np.float32).copy()
    # fold tanh->sigmoid (2x on g-gate inputs), and 2x on all of Wh to
    # compensate h' = h/2 stored on-chip.
    Wx[:, 512:768] *= 2.0
    b[512:768] *= 2.0
    Wh *= 2.0
    Wh[:, 512:768] *= 2.0

    whT = np.empty((128, 16 * 128), np.float32)
    for m in range(8):
        for k in range(2):
            whT[:, (m * 2 + k) * 128:(m * 2 + k + 1) * 128] = \
                Wh[k * 128:(k + 1) * 128, m * 128:(m + 1) * 128]
    wxT = np.empty((128, 8 * 128), np.float32)
    for m in range(8):
        wxT[:, m * 128:(m + 1) * 128] = Wx[:, m * 128:(m + 1) * 128]
    bT = b.reshape(8, 128).T.copy()
    wdT = np.empty((128, 8), np.float32)
    for k in range(2):
        wdT[:, k * 4:(k + 1) * 4] = Wd[d * 256 + k * 128:
                                       d * 256 + (k + 1) * 128, :]

    it = np.arange(N_ITERS)[:, None, None]
    p = np.arange(128)[None, :, None]
    cj = np.arange(2 * 4)[None, None, :]
    chain, j = cj // 4, cj % 4
    s_local = j * 4 + p // 32
    jb = p % 32
    t = it * STEPS + s_local
    if d == 1:
        t = (T_FULL - 1) - t
    row = s * 64 + chain * B + jb
    idx = np.ascontiguousarray(x[row, t].astype(np.int32))

    return {
        "emb": emb_np,
        "whT": np.ascontiguousarray(whT.astype(W_NP)),
        "wxT": np.ascontiguousarray(wxT.astype(W_NP)),
        "bT": np.ascontiguousarray(bT),
        "wdT": wdT,
        "identf": np.eye(128, dtype=np.float32),
        "identw": np.eye(128).astype(W_NP),
        "idx": idx,
    }


def kernel(x, train, embed_table, Wx_f, Wh_f, b_f, Wx_b, Wh_b, b_b, Wd, bd,
           **_unused):
    from concourse.bass_utils import run_bass_kernel_spmd

    x = np.asarray(x).astype(np.int64)
    emb_np = np.ascontiguousarray(np.asarray(embed_table, np.float32))
    Wd_np = np.asarray(Wd, np.float32)

    if "nc" not in _CACHE:
        _CACHE["nc"] = _build_program()
    nc = _CACHE["nc"]

    in_maps = []
    for core in range(N_CORES):
        if core < 4:
            Wx, Wh, b = Wx_f, Wh_f, b_f
        else:
            Wx, Wh, b = Wx_b, Wh_b, b_b
        in_maps.append(_prep_core_inputs(
            core, x, emb_np, np.asarray(Wx), np.asarray(Wh), np.asarray(b),
            Wd_np))

    res = run_bass_kernel_spmd(nc, in_maps, list(range(N_CORES))).results

    logits = np.zeros((B_FULL, NUM_CLASSES), np.float32)
    for core in range(N_CORES):
        s = core % 4
        o = np.asarray(res[core]["out"], np.float32)  # [CHAINS, 4, B]
        for c in range(CHAINS):
            r0 = s * 64 + c * B
            logits[r0:r0 + B] += o[c].T
    logits += np.asarray(bd, np.float32)[None, :]
    return logits
